# revision 5
# baseline (speedup 1.0000x reference)
"""Trainium2 Bass kernel for nn_MinibatchDiscrimination (B=256, F=1024, O=128, K=8).

Computes out = concat([x, c], axis=1) where
    M = (x @ T.reshape(F, O*K)).reshape(B, O, K)
    c[i, o] = sum_{j != i} exp(-sum_k |M[j,o,k] - M[i,o,k]|)
(the reference's `sum_j exp(-d) - 1` equals the self-term-excluded sum since
 d_ii == 0 exactly).

Distribution: batch rows of c are sharded across 8 cores (32 rows each).
Every core redundantly computes the full GEMM (it needs all of M for the
pairwise reduction anyway; the GEMM is ~7% of the work).  SPMD divergence is
achieved purely through data: core b receives x^T with its batch columns
rotated so that its 32 owned rows sit at local columns [0, 32) — the pairwise
sum over j is permutation-invariant, so compile-time index offsets work for
every core.

Per-core device pipeline (all layouts use partition p = (o%64)*2 + (k%2),
column group g = 4*(o//64) + ((k//2)%4), i.e. 2 o-halves x 4 k-quarters;
matmul outputs may only be placed at partition offsets {0, 64}):
  1. DMA  w (bf16 F x O*K), xt (bf16 F x B, rolled), mask (bf16 128x32).
  2. GEMM Mt[p, g*B + j] = M[j, o(p,g), k(p,g)]   (TensorE, bf16, f32 PSUM),
     copied to SBUF as bf16 (full) + f32 (first 32 columns, tensor_scalar
     operands must be f32).
  2b. S[o, j] = sum_k M[j,o,k] via a mask-matmul over Mt; stored as
     -S/2 (f32, per (o,j)) and -S[:, :32] (f32 bias columns).
  3. For each local row i in [0,32), using |d| = 2*relu(d) - d and
     sum_k d_k = S_j - S_i:
     a. VectorE tensor_scalar:  R = relu(Mt - Mt[:, i])  (fused
        subtract+max-0, bf16, 4x mode), 8 column groups.
     b. ScalarE prefills the PSUM tile with -S_j/2, then TensorE mask-matmuls
        accumulate (start=False): h[o, j] = sum_k relu(d_k) - S[o,j]/2.
     c. ScalarE activation Exp(scale=-2, bias=-S[:, i]) with accum_out ->
        c_acc[:, i] = sum_j exp(-2h - S_i) = sum_j exp(-diffs);
        the self term e[:, i] is extracted to c_diag[:, i] (this also keeps
        the diagonal cancellation exact).
  4. c = c_acc - c_diag (VectorE), DMA out as (O, 32) f32.
Host gathers: c_full rows [32b, 32b+32) = core_b_out.T; output = [x | c_full].
"""

import numpy as np
import ml_dtypes

B, F, O, K = 256, 1024, 128, 8
NCORES = 8
IB = B // NCORES  # c rows owned per core

_cache = {}


def _build():
    from contextlib import ExitStack
    import concourse.bacc as bacc
    import concourse.tile as tile
    import concourse.mybir as mybir

    dt = mybir.dt
    Alu = mybir.AluOpType
    Act = mybir.ActivationFunctionType

    nc = bacc.Bacc(
        "TRN2", target_bir_lowering=False, debug=False, enable_asserts=False
    )
    w = nc.dram_tensor("w", (F, O * K), dt.bfloat16, kind="ExternalInput").ap()
    xt = nc.dram_tensor("xt", (F, B), dt.bfloat16, kind="ExternalInput").ap()
    mask = nc.dram_tensor("mask", (128, 64), dt.bfloat16, kind="ExternalInput").ap()
    cout = nc.dram_tensor("c", (O, IB), dt.float32, kind="ExternalOutput").ap()

    FC = F // 128  # contraction chunks
    G = 8  # column groups (2 o-halves x 4 k-quarters)

    with ExitStack() as ctx:
        tc = ctx.enter_context(tile.TileContext(nc))
        inpool = ctx.enter_context(tc.tile_pool(name="inp", bufs=1))
        mpool = ctx.enter_context(tc.tile_pool(name="mt", bufs=1))
        dpool = ctx.enter_context(tc.tile_pool(name="d", bufs=3))
        cpool = ctx.enter_context(tc.tile_pool(name="c", bufs=1))

        mask_sb = inpool.tile([128, 64], dt.bfloat16, tag="mask")
        nc.sync.dma_start(mask_sb[:], mask)
        w_tiles, x_tiles = [], []
        for fc in range(FC):
            wt = inpool.tile([128, O * K], dt.bfloat16, tag=f"w{fc}", name=f"w{fc}")
            nc.sync.dma_start(wt[:], w[fc * 128 : (fc + 1) * 128, :])
            w_tiles.append(wt)
            xtt = inpool.tile([128, B], dt.bfloat16, tag=f"x{fc}", name=f"x{fc}")
            nc.sync.dma_start(xtt[:], xt[fc * 128 : (fc + 1) * 128, :])
            x_tiles.append(xtt)

        # Mt: partition p=(o%32)*4+k%4, free = (g, j); g = 2*(o//32) + k//4
        mt = mpool.tile([128, G * B], dt.bfloat16, tag="mt")
        # f32 copy of the first IB columns of each group (tensor_scalar scalars)
        mtf = mpool.tile([128, G * IB], dt.float32, tag="mtf")

        with tc.tile_pool(name="gps", bufs=1, space="PSUM") as gps:
            gm = [gps.tile([128, B], dt.float32, tag=f"g{i}", name=f"g{i}") for i in range(G)]
            for fc in range(FC):
                # host pre-permuted w: col = (g*64 + o%64)*2 + k%2, g=4*(o//64)+k//2
                for g in range(G):
                    nc.tensor.matmul(
                        gm[g][:],
                        w_tiles[fc][:, g * 128 : (g + 1) * 128],
                        x_tiles[fc][:],
                        start=(fc == 0),
                        stop=(fc == FC - 1),
                    )
            for g in range(G):
                nc.scalar.copy(mt[:, g * B : (g + 1) * B], gm[g][:])
            for g in range(G):
                nc.scalar.copy(
                    mtf[:, g * IB : (g + 1) * IB], mt[:, g * B : g * B + IB]
                )

        # S[o, j] = sum_k M[j, o, k] via mask-matmul on Mt
        neg_s_half = mpool.tile([128, B], dt.float32, tag="negsh")  # -S/2
        neg_s_bias = mpool.tile([128, IB], dt.float32, tag="negsb")  # -S[:, :IB]
        with tc.tile_pool(name="sps", bufs=1, space="PSUM") as spsp:
            s_ps = spsp.tile([128, B], dt.float32, tag="sps")
            for q in range(2):
                for kh in range(4):
                    g = 4 * q + kh
                    nc.tensor.matmul(
                        s_ps[64 * q : 64 * (q + 1), :],
                        mask_sb[:],
                        mt[:, g * B : (g + 1) * B],
                        start=(kh == 0),
                        stop=(kh == 3),
                    )
            nc.scalar.mul(neg_s_half[:], s_ps[:], -0.5)
            nc.scalar.mul(neg_s_bias[:], s_ps[:, 0:IB], -1.0)

        c_acc = cpool.tile([128, IB], dt.float32, tag="cacc")
        c_diag = cpool.tile([128, IB], dt.float32, tag="cdiag")
        c_fin = cpool.tile([128, IB], dt.float32, tag="cfin")

        with tc.tile_pool(name="dps", bufs=2, space="PSUM") as dps, tc.tile_pool(
            name="eps", bufs=2, space="PSUM"
        ) as eps:
            for i in range(IB):
                d = dpool.tile([128, G * B], dt.bfloat16, tag="d")
                for g in range(G):
                    nc.vector.tensor_scalar(
                        d[:, g * B : (g + 1) * B],
                        mt[:, g * B : (g + 1) * B],
                        mtf[:, g * IB + i : g * IB + i + 1],
                        0.0,
                        Alu.subtract,
                        Alu.max,
                    )
                diffs = dps.tile([128, B], dt.float32, tag="diffs")
                nc.scalar.copy(diffs[:], neg_s_half[:])
                for q in range(2):
                    for kh in range(4):
                        g = 4 * q + kh
                        nc.tensor.matmul(
                            diffs[64 * q : 64 * (q + 1), :],
                            mask_sb[:],
                            d[:, g * B : (g + 1) * B],
                            start=False,
                            stop=(kh == 3),
                            skip_group_check=True,
                        )
                e = eps.tile([128, B], dt.float32, tag="e")
                nc.scalar.activation(
                    e[:],
                    diffs[:],
                    Act.Exp,
                    scale=-2.0,
                    bias=neg_s_bias[:, i : i + 1],
                    accum_out=c_acc[:, i : i + 1],
                )
                nc.scalar.copy(c_diag[:, i : i + 1], e[:, i : i + 1])

        nc.vector.tensor_tensor(c_fin[:], c_acc[:], c_diag[:], op=Alu.subtract)
        nc.sync.dma_start(cout, c_fin[:])

    nc.compile()
    return nc


def _prep_inputs(x, T):
    bf16 = ml_dtypes.bfloat16
    # permute W columns to (q, kh, o%64, k%2) so every per-group lhsT slice of
    # the GEMM is contiguous (walrus: weights AP must have one free dim)
    Wp = (
        np.asarray(T, np.float32)
        .reshape(F, 2, 64, 4, 2)
        .transpose(0, 1, 3, 2, 4)
        .reshape(F, O * K)
    )
    W = np.ascontiguousarray(Wp).astype(bf16)
    xT = np.ascontiguousarray(x.T.astype(np.float32)).astype(bf16)
    mask = np.zeros((128, 64), dtype=bf16)
    mask[np.arange(128), np.arange(128) // 2] = 1.0
    in_maps = []
    for b in range(NCORES):
        in_maps.append(
            {
                "w": W,
                "mask": mask,
                "xt": np.ascontiguousarray(np.roll(xT, -IB * b, axis=1)),
            }
        )
    return in_maps


def _assemble(x, results):
    c_full = np.concatenate([r["c"].T for r in results], axis=0)  # (B, O)
    return np.concatenate(
        [np.asarray(x, np.float32), c_full.astype(np.float32)], axis=1
    )


def _get_nc():
    if "nc" not in _cache:
        _cache["nc"] = _build()
    return _cache["nc"]


def kernel(x, T):
    from concourse.bass_utils import run_bass_kernel_spmd

    x = np.asarray(x)
    T = np.asarray(T)
    nc = _get_nc()
    res = run_bass_kernel_spmd(nc, _prep_inputs(x, T), list(range(NCORES)))
    return _assemble(x, res.results)


def run_traced(x, T, **kwargs):
    """Like kernel() but returns (output, BassKernelResults) with tracing on."""
    from concourse.bass_utils import run_bass_kernel_spmd

    x = np.asarray(x)
    T = np.asarray(T)
    nc = _get_nc()
    res = run_bass_kernel_spmd(
        nc, _prep_inputs(x, T), list(range(NCORES)), trace=True, **kwargs
    )
    return _assemble(x, res.results), res


# revision 13
# speedup vs baseline: 1.0163x; 1.0163x over previous
"""Trainium2 Bass kernel for nn_MinibatchDiscrimination (B=256, F=1024, O=128, K=8).

Computes out = concat([x, c], axis=1) where
    M = (x @ T.reshape(F, O*K)).reshape(B, O, K)
    c[i, o] = sum_{j != i} exp(-sum_k |M[j,o,k] - M[i,o,k]|)
(the reference's `sum_j exp(-d) - 1` equals the self-term-excluded sum since
 d_ii == 0 exactly).

Distribution: batch rows of c are sharded across 8 cores (32 rows each).
Every core redundantly computes the full GEMM (it needs all of M for the
pairwise reduction anyway; the GEMM is ~7% of the work).  SPMD divergence is
achieved purely through data: core b receives x^T with its batch columns
rotated so that its 32 owned rows sit at local columns [0, 32) — the pairwise
sum over j is permutation-invariant, so compile-time index offsets work for
every core.

Per-core device pipeline (all layouts use partition p = (o%64)*2 + (k%2),
column group g = 4*(o//64) + ((k//2)%4), i.e. 2 o-halves x 4 k-quarters;
matmul outputs may only be placed at partition offsets {0, 64}):
  1. DMA  w (bf16 F x O*K), xt (bf16 F x B, rolled), mask (bf16 128x32).
  2. GEMM Mt[p, g*B + j] = M[j, o(p,g), k(p,g)]   (TensorE, bf16, f32 PSUM),
     copied to SBUF as bf16 (full) + f32 (first 32 columns, tensor_scalar
     operands must be f32).
  2b. S[o, j] = sum_k M[j,o,k] via a mask-matmul over Mt; stored as
     -S/2 (f32, per (o,j)) and -S[:, :32] (f32 bias columns).
  3. For each local row i in [0,32), using |d| = 2*relu(d) - d and
     sum_k d_k = S_j - S_i:
     a. VectorE tensor_scalar:  R = relu(Mt - Mt[:, i])  (fused
        subtract+max-0, bf16, 4x mode), 8 column groups.
     b. ScalarE prefills the PSUM tile with -S_j/2, then TensorE mask-matmuls
        accumulate (start=False): h[o, j] = sum_k relu(d_k) - S[o,j]/2.
     c. ScalarE activation Exp(scale=-2, bias=-S[:, i]) with accum_out ->
        c_acc[:, i] = sum_j exp(-2h - S_i) = sum_j exp(-diffs);
        the self term e[:, i] is extracted to c_diag[:, i] (this also keeps
        the diagonal cancellation exact).
  4. c = c_acc - c_diag (VectorE), DMA out as (O, 32) f32.
Host gathers: c_full rows [32b, 32b+32) = core_b_out.T; output = [x | c_full].
"""

import numpy as np
import ml_dtypes

B, F, O, K = 256, 1024, 128, 8
NCORES = 8
IB = B // NCORES  # c rows owned per core

_cache = {}


def _build():
    from contextlib import ExitStack
    import concourse.bacc as bacc
    import concourse.tile as tile
    import concourse.mybir as mybir

    dt = mybir.dt
    Alu = mybir.AluOpType
    Act = mybir.ActivationFunctionType

    nc = bacc.Bacc(
        "TRN2", target_bir_lowering=False, debug=False, enable_asserts=False
    )
    w = nc.dram_tensor("w", (F, O * K), dt.bfloat16, kind="ExternalInput").ap()
    xt = nc.dram_tensor("xt", (F, B), dt.bfloat16, kind="ExternalInput").ap()
    mask = nc.dram_tensor("mask", (128, 64), dt.bfloat16, kind="ExternalInput").ap()
    dmask = nc.dram_tensor("dmask", (128, 128), dt.bfloat16, kind="ExternalInput").ap()
    cout = nc.dram_tensor("c", (O, IB), dt.float32, kind="ExternalOutput").ap()

    FC = F // 128  # contraction chunks
    G = 8  # column groups (2 o-halves x 4 k-quarters)

    with ExitStack() as ctx:
        tc = ctx.enter_context(tile.TileContext(nc))
        inpool = ctx.enter_context(tc.tile_pool(name="inp", bufs=1))
        mpool = ctx.enter_context(tc.tile_pool(name="mt", bufs=1))
        dpool = ctx.enter_context(tc.tile_pool(name="d", bufs=3))
        cpool = ctx.enter_context(tc.tile_pool(name="c", bufs=1))

        mask_sb = inpool.tile([128, 64], dt.bfloat16, tag="mask")
        nc.sync.dma_start(mask_sb[:], mask)
        dmask_sb = inpool.tile([128, 128], dt.bfloat16, tag="dmask")
        nc.sync.dma_start(dmask_sb[:], dmask)
        w_tiles, x_tiles = [], []
        for fc in range(FC):
            wt = inpool.tile([128, O * K], dt.bfloat16, tag=f"w{fc}", name=f"w{fc}")
            nc.sync.dma_start(wt[:], w[fc * 128 : (fc + 1) * 128, :])
            w_tiles.append(wt)
            xtt = inpool.tile([128, B], dt.bfloat16, tag=f"x{fc}", name=f"x{fc}")
            nc.sync.dma_start(xtt[:], xt[fc * 128 : (fc + 1) * 128, :])
            x_tiles.append(xtt)

        # Mt: partition p=(o%64)*2+k%2, free j; one tile per group g = 4*(o//64)+k//2
        mt = [
            mpool.tile([128, B], dt.bfloat16, tag=f"mt{g}", name=f"mt{g}")
            for g in range(G)
        ]
        # f32 copies of the first IB columns of each group (tensor_scalar scalars)
        mtf = [
            mpool.tile([128, IB], dt.float32, tag=f"mtf{g}", name=f"mtf{g}")
            for g in range(G)
        ]
        # negated f32 columns (activation-Abs bias for the ScalarE row variant)
        nmtf = [
            mpool.tile([128, IB], dt.float32, tag=f"nmtf{g}", name=f"nmtf{g}")
            for g in range(G)
        ]

        with tc.tile_pool(name="gps", bufs=1, space="PSUM") as gps:
            gm = [gps.tile([128, B], dt.float32, tag=f"g{i}", name=f"g{i}") for i in range(G)]
            for fc in range(FC):
                # host pre-permuted w: col = (g*64 + o%64)*2 + k%2, g=4*(o//64)+k//2
                for g in range(G):
                    nc.tensor.matmul(
                        gm[g][:],
                        w_tiles[fc][:, g * 128 : (g + 1) * 128],
                        x_tiles[fc][:],
                        start=(fc == 0),
                        stop=(fc == FC - 1),
                    )
            for g in range(G):
                nc.scalar.copy(mt[g][:], gm[g][:])
            for g in range(G):
                nc.scalar.copy(mtf[g][:], mt[g][:, 0:IB])
                nc.scalar.mul(nmtf[g][:], mt[g][:, 0:IB], -1.0)

        # S[o, j] = sum_k M[j, o, k].  Two forms are kept:
        #  - sneg[q]: -S/4 in the duplicated (o%64)*2+k%2 partition layout
        #    (bf16): streamed as the first pairwise matmul of each row with
        #    start=True, so the mask-sum over partition pairs seeds the PSUM
        #    with -S_j/2.
        #  - neg_s_bias: -bf16(S) as f32 in o-layout: the per-partition exp
        #    bias.  bf16(S/4)*4 == bf16(S) exactly, so the diagonal argument
        #    cancels to exactly 0 for every row and e[:, i] is one constant
        #    column, extracted once (c_diag0).
        sneg = [
            mpool.tile([128, B], dt.bfloat16, tag=f"sneg{q}", name=f"sneg{q}")
            for q in range(2)
        ]
        s_bf = mpool.tile([128, IB], dt.bfloat16, tag="sbf")
        neg_s_bias = mpool.tile([128, IB], dt.float32, tag="negsb")
        with tc.tile_pool(name="sps", bufs=1, space="PSUM") as spsp:
            s_ps = spsp.tile([128, B], dt.float32, tag="sps")
            for q in range(2):
                for kh in range(4):
                    g = 4 * q + kh
                    nc.tensor.matmul(
                        s_ps[64 * q : 64 * (q + 1), :],
                        mask_sb[:],
                        mt[g][:],
                        start=(kh == 0),
                        stop=(kh == 3),
                    )
            nc.scalar.copy(s_bf[:], s_ps[:, 0:IB])
            nc.scalar.mul(neg_s_bias[:], s_bf[:], -1.0)
            sd_ps = spsp.tile([128, B], dt.float32, tag="sdps")
            for q in range(2):
                for kh in range(4):
                    g = 4 * q + kh
                    nc.tensor.matmul(
                        sd_ps[:],
                        dmask_sb[:],
                        mt[g][:],
                        start=(kh == 0),
                        stop=(kh == 3),
                    )
                nc.scalar.mul(sneg[q][:], sd_ps[:], -0.25)

        c_acc = cpool.tile([128, IB], dt.float32, tag="cacc")
        c_diag = cpool.tile([128, IB], dt.float32, tag="cdiag")
        c_fin = cpool.tile([128, IB], dt.float32, tag="cfin")

        with tc.tile_pool(name="dps", bufs=2, space="PSUM") as dps, tc.tile_pool(
            name="eps", bufs=2, space="PSUM"
        ) as eps:
            first_mm = True
            for i in range(IB):
                act_row = i % 8 == 7  # rows computed via ScalarE Abs (balance)
                d = dpool.tile([128, G * B], dt.bfloat16, tag="d")
                for g in range(G):
                    if act_row:
                        nc.scalar.activation(
                            d[:, g * B : (g + 1) * B],
                            mt[g][:],
                            Act.Abs,
                            bias=nmtf[g][:, i : i + 1],
                        )
                    else:
                        nc.vector.tensor_scalar(
                            d[:, g * B : (g + 1) * B],
                            mt[g][:],
                            mtf[g][:, i : i + 1],
                            0.0,
                            Alu.subtract,
                            Alu.max,
                        )
                diffs = dps.tile([128, B], dt.float32, tag="diffs")
                for q in range(2):
                    mms = []
                    if not act_row:
                        # seed the PSUM quadrant with -S_j/2
                        mms.append(
                            nc.tensor.matmul(
                                diffs[64 * q : 64 * (q + 1), :],
                                mask_sb[:],
                                sneg[q][:],
                                start=True,
                                stop=False,
                                skip_group_check=True,
                            )
                        )
                    for kh in range(4):
                        g = 4 * q + kh
                        mms.append(
                            nc.tensor.matmul(
                                diffs[64 * q : 64 * (q + 1), :],
                                mask_sb[:],
                                d[:, g * B : (g + 1) * B],
                                start=(act_row and kh == 0),
                                stop=(kh == 3),
                                skip_group_check=True,
                            )
                        )
                    for mm in mms:
                        # every pairwise matmul reuses the identical mask
                        # weights: skip the LDWEIGHTS reload after the first
                        if not first_mm:
                            mm.ins.ldweights = False
                        first_mm = False
                e = eps.tile([128, B], dt.float32, tag="e")
                nc.scalar.activation(
                    e[:],
                    diffs[:],
                    Act.Exp,
                    scale=(-1.0 if act_row else -2.0),
                    bias=(0.0 if act_row else neg_s_bias[:, i : i + 1]),
                    accum_out=c_acc[:, i : i + 1],
                )
                if i == 0:
                    # diagonal argument is exactly 0 for every row (see S
                    # handling above), so one extracted column serves all
                    nc.scalar.copy(c_diag[:, 0:1], e[:, 0:1])

        nc.vector.tensor_scalar_sub(c_fin[:], c_acc[:], c_diag[:, 0:1])
        nc.sync.dma_start(cout, c_fin[:])

    nc.compile()
    return nc


def _prep_inputs(x, T):
    bf16 = ml_dtypes.bfloat16
    # permute W columns to (q, kh, o%64, k%2) so every per-group lhsT slice of
    # the GEMM is contiguous (walrus: weights AP must have one free dim)
    Wp = (
        np.asarray(T, np.float32)
        .reshape(F, 2, 64, 4, 2)
        .transpose(0, 1, 3, 2, 4)
        .reshape(F, O * K)
    )
    W = np.ascontiguousarray(Wp).astype(bf16)
    xT = np.ascontiguousarray(x.T.astype(np.float32)).astype(bf16)
    mask = np.zeros((128, 64), dtype=bf16)
    mask[np.arange(128), np.arange(128) // 2] = 1.0
    dmask = np.zeros((128, 128), dtype=bf16)
    p = np.arange(128)
    dmask[:, :] = (p[:, None] // 2 == p[None, :] // 2).astype(bf16)
    in_maps = []
    for b in range(NCORES):
        in_maps.append(
            {
                "w": W,
                "mask": mask,
                "dmask": dmask,
                "xt": np.ascontiguousarray(np.roll(xT, -IB * b, axis=1)),
            }
        )
    return in_maps


def _assemble(x, results):
    c_full = np.concatenate([r["c"].T for r in results], axis=0)  # (B, O)
    return np.concatenate(
        [np.asarray(x, np.float32), c_full.astype(np.float32)], axis=1
    )


def _get_nc():
    if "nc" not in _cache:
        _cache["nc"] = _build()
    return _cache["nc"]


def kernel(x, T):
    from concourse.bass_utils import run_bass_kernel_spmd

    x = np.asarray(x)
    T = np.asarray(T)
    nc = _get_nc()
    res = run_bass_kernel_spmd(nc, _prep_inputs(x, T), list(range(NCORES)))
    return _assemble(x, res.results)


def run_traced(x, T, **kwargs):
    """Like kernel() but returns (output, BassKernelResults) with tracing on."""
    from concourse.bass_utils import run_bass_kernel_spmd

    x = np.asarray(x)
    T = np.asarray(T)
    nc = _get_nc()
    res = run_bass_kernel_spmd(
        nc, _prep_inputs(x, T), list(range(NCORES)), trace=True, **kwargs
    )
    return _assemble(x, res.results), res


# revision 14
# speedup vs baseline: 1.0224x; 1.0060x over previous
"""Trainium2 Bass kernel for nn_MinibatchDiscrimination (B=256, F=1024, O=128, K=8).

Computes out = concat([x, c], axis=1) where
    M = (x @ T.reshape(F, O*K)).reshape(B, O, K)
    c[i, o] = sum_{j != i} exp(-sum_k |M[j,o,k] - M[i,o,k]|)
(the reference's `sum_j exp(-d) - 1` equals the self-term-excluded sum since
 d_ii == 0 exactly).

Distribution: batch rows of c are sharded across 8 cores (32 rows each).
Every core redundantly computes the full GEMM (it needs all of M for the
pairwise reduction anyway; the GEMM is ~7% of the work).  SPMD divergence is
achieved purely through data: core b receives x^T with its batch columns
rotated so that its 32 owned rows sit at local columns [0, 32) — the pairwise
sum over j is permutation-invariant, so compile-time index offsets work for
every core.

Per-core device pipeline (all layouts use partition p = (o%64)*2 + (k%2),
column group g = 4*(o//64) + ((k//2)%4), i.e. 2 o-halves x 4 k-quarters;
matmul outputs may only be placed at partition offsets {0, 64}):
  1. DMA  w (bf16 F x O*K), xt (bf16 F x B, rolled), mask (bf16 128x32).
  2. GEMM Mt[p, g*B + j] = M[j, o(p,g), k(p,g)]   (TensorE, bf16, f32 PSUM),
     copied to SBUF as bf16 (full) + f32 (first 32 columns, tensor_scalar
     operands must be f32).
  2b. S[o, j] = sum_k M[j,o,k] via a mask-matmul over Mt; stored as
     -S/2 (f32, per (o,j)) and -S[:, :32] (f32 bias columns).
  3. For each local row i in [0,32), using |d| = 2*relu(d) - d and
     sum_k d_k = S_j - S_i:
     a. VectorE tensor_scalar:  R = relu(Mt - Mt[:, i])  (fused
        subtract+max-0, bf16, 4x mode), 8 column groups.
     b. ScalarE prefills the PSUM tile with -S_j/2, then TensorE mask-matmuls
        accumulate (start=False): h[o, j] = sum_k relu(d_k) - S[o,j]/2.
     c. ScalarE activation Exp(scale=-2, bias=-S[:, i]) with accum_out ->
        c_acc[:, i] = sum_j exp(-2h - S_i) = sum_j exp(-diffs);
        the self term e[:, i] is extracted to c_diag[:, i] (this also keeps
        the diagonal cancellation exact).
  4. c = c_acc - c_diag (VectorE), DMA out as (O, 32) f32.
Host gathers: c_full rows [32b, 32b+32) = core_b_out.T; output = [x | c_full].
"""

import numpy as np
import ml_dtypes

B, F, O, K = 256, 1024, 128, 8
NCORES = 8
IB = B // NCORES  # c rows owned per core

_cache = {}


def _build():
    from contextlib import ExitStack
    import concourse.bacc as bacc
    import concourse.tile as tile
    import concourse.mybir as mybir

    dt = mybir.dt
    Alu = mybir.AluOpType
    Act = mybir.ActivationFunctionType

    nc = bacc.Bacc(
        "TRN2", target_bir_lowering=False, debug=False, enable_asserts=False
    )
    w = nc.dram_tensor("w", (F, O * K), dt.float8e4, kind="ExternalInput").ap()
    xt = nc.dram_tensor("xt", (F, B), dt.float8e4, kind="ExternalInput").ap()
    mask = nc.dram_tensor("mask", (128, 64), dt.bfloat16, kind="ExternalInput").ap()
    dmask = nc.dram_tensor("dmask", (128, 128), dt.bfloat16, kind="ExternalInput").ap()
    cout = nc.dram_tensor("c", (O, IB), dt.float32, kind="ExternalOutput").ap()

    FC = F // 128  # contraction chunks
    G = 8  # column groups (2 o-halves x 4 k-quarters)

    with ExitStack() as ctx:
        tc = ctx.enter_context(tile.TileContext(nc))
        inpool = ctx.enter_context(tc.tile_pool(name="inp", bufs=1))
        mpool = ctx.enter_context(tc.tile_pool(name="mt", bufs=1))
        dpool = ctx.enter_context(tc.tile_pool(name="d", bufs=3))
        cpool = ctx.enter_context(tc.tile_pool(name="c", bufs=1))

        w_tiles, x_tiles = [], []
        for fc in range(FC):
            xtt = inpool.tile([128, B], dt.float8e4, tag=f"x{fc}", name=f"x{fc}")
            nc.sync.dma_start(xtt[:], xt[fc * 128 : (fc + 1) * 128, :])
            x_tiles.append(xtt)
            wt = inpool.tile([128, O * K], dt.float8e4, tag=f"w{fc}", name=f"w{fc}")
            nc.sync.dma_start(wt[:], w[fc * 128 : (fc + 1) * 128, :])
            w_tiles.append(wt)
        mask_sb = inpool.tile([128, 64], dt.bfloat16, tag="mask")
        nc.sync.dma_start(mask_sb[:], mask)
        dmask_sb = inpool.tile([128, 128], dt.bfloat16, tag="dmask")
        nc.sync.dma_start(dmask_sb[:], dmask)

        # Mt: partition p=(o%64)*2+k%2, free j; one tile per group g = 4*(o//64)+k//2
        mt = [
            mpool.tile([128, B], dt.bfloat16, tag=f"mt{g}", name=f"mt{g}")
            for g in range(G)
        ]
        # f32 copies of the first IB columns of each group (tensor_scalar scalars)
        mtf = [
            mpool.tile([128, IB], dt.float32, tag=f"mtf{g}", name=f"mtf{g}")
            for g in range(G)
        ]
        # negated f32 columns (activation-Abs bias for the ScalarE row variant)
        nmtf = [
            mpool.tile([128, IB], dt.float32, tag=f"nmtf{g}", name=f"nmtf{g}")
            for g in range(G)
        ]

        with tc.tile_pool(name="gps", bufs=2, space="PSUM") as gps:
            # host pre-permuted w: col = (g*64 + o%64)*2 + k%2, g=4*(o//64)+k//2
            # g-outer so mt[0] (and the pairwise loop) starts as soon as the
            # last w/x chunk lands rather than after the whole GEMM
            for g in range(G):
                gm = gps.tile([128, B], dt.float32, tag="gm", name=f"gm{g}")
                for fc in range(FC):
                    nc.tensor.matmul(
                        gm[:],
                        w_tiles[fc][:, g * 128 : (g + 1) * 128],
                        x_tiles[fc][:],
                        start=(fc == 0),
                        stop=(fc == FC - 1),
                    )
                nc.scalar.copy(mt[g][:], gm[:])
                nc.scalar.copy(mtf[g][:], mt[g][:, 0:IB])
                nc.scalar.mul(nmtf[g][:], mt[g][:, 0:IB], -1.0)

        # S[o, j] = sum_k M[j, o, k].  Two forms are kept:
        #  - sneg[q]: -S/4 in the duplicated (o%64)*2+k%2 partition layout
        #    (bf16): streamed as the first pairwise matmul of each row with
        #    start=True, so the mask-sum over partition pairs seeds the PSUM
        #    with -S_j/2.
        #  - neg_s_bias: -bf16(S) as f32 in o-layout: the per-partition exp
        #    bias.  bf16(S/4)*4 == bf16(S) exactly, so the diagonal argument
        #    cancels to exactly 0 for every row and e[:, i] is one constant
        #    column, extracted once (c_diag0).
        sneg = [
            mpool.tile([128, B], dt.bfloat16, tag=f"sneg{q}", name=f"sneg{q}")
            for q in range(2)
        ]
        s_bf = mpool.tile([128, IB], dt.bfloat16, tag="sbf")
        neg_s_bias = mpool.tile([128, IB], dt.float32, tag="negsb")
        with tc.tile_pool(name="sps", bufs=1, space="PSUM") as spsp:
            s_ps = spsp.tile([128, B], dt.float32, tag="sps")
            for q in range(2):
                for kh in range(4):
                    g = 4 * q + kh
                    nc.tensor.matmul(
                        s_ps[64 * q : 64 * (q + 1), :],
                        mask_sb[:],
                        mt[g][:],
                        start=(kh == 0),
                        stop=(kh == 3),
                    )
            nc.scalar.copy(s_bf[:], s_ps[:, 0:IB])
            nc.scalar.mul(neg_s_bias[:], s_bf[:], -1.0)
            sd_ps = spsp.tile([128, B], dt.float32, tag="sdps")
            for q in range(2):
                for kh in range(4):
                    g = 4 * q + kh
                    nc.tensor.matmul(
                        sd_ps[:],
                        dmask_sb[:],
                        mt[g][:],
                        start=(kh == 0),
                        stop=(kh == 3),
                    )
                nc.scalar.mul(sneg[q][:], sd_ps[:], -0.25)

        c_acc = cpool.tile([128, IB], dt.float32, tag="cacc")
        c_diag = cpool.tile([128, IB], dt.float32, tag="cdiag")
        c_fin = cpool.tile([128, IB], dt.float32, tag="cfin")

        with tc.tile_pool(name="dps", bufs=2, space="PSUM") as dps, tc.tile_pool(
            name="eps", bufs=2, space="PSUM"
        ) as eps:
            first_mm = True
            for i in range(IB):
                act_row = i % 8 == 7  # rows computed via ScalarE Abs (balance)
                d = dpool.tile([128, G * B], dt.bfloat16, tag="d")
                for g in range(G):
                    if act_row:
                        nc.scalar.activation(
                            d[:, g * B : (g + 1) * B],
                            mt[g][:],
                            Act.Abs,
                            bias=nmtf[g][:, i : i + 1],
                        )
                    else:
                        nc.vector.tensor_scalar(
                            d[:, g * B : (g + 1) * B],
                            mt[g][:],
                            mtf[g][:, i : i + 1],
                            0.0,
                            Alu.subtract,
                            Alu.max,
                        )
                diffs = dps.tile([128, B], dt.float32, tag="diffs")
                for q in range(2):
                    mms = []
                    if not act_row:
                        # seed the PSUM quadrant with -S_j/2
                        mms.append(
                            nc.tensor.matmul(
                                diffs[64 * q : 64 * (q + 1), :],
                                mask_sb[:],
                                sneg[q][:],
                                start=True,
                                stop=False,
                                skip_group_check=True,
                            )
                        )
                    for kh in range(4):
                        g = 4 * q + kh
                        mms.append(
                            nc.tensor.matmul(
                                diffs[64 * q : 64 * (q + 1), :],
                                mask_sb[:],
                                d[:, g * B : (g + 1) * B],
                                start=(act_row and kh == 0),
                                stop=(kh == 3),
                                skip_group_check=True,
                            )
                        )
                    for mm in mms:
                        # every pairwise matmul reuses the identical mask
                        # weights: skip the LDWEIGHTS reload after the first
                        if not first_mm:
                            mm.ins.ldweights = False
                        first_mm = False
                e = eps.tile([128, B], dt.float32, tag="e")
                nc.scalar.activation(
                    e[:],
                    diffs[:],
                    Act.Exp,
                    scale=(-1.0 if act_row else -2.0),
                    bias=(0.0 if act_row else neg_s_bias[:, i : i + 1]),
                    accum_out=c_acc[:, i : i + 1],
                )
                if i == 0:
                    # diagonal argument is exactly 0 for every row (see S
                    # handling above), so one extracted column serves all
                    nc.scalar.copy(c_diag[:, 0:1], e[:, 0:1])

        nc.vector.tensor_scalar_sub(c_fin[:], c_acc[:], c_diag[:, 0:1])
        nc.sync.dma_start(cout, c_fin[:])

    nc.compile()
    return nc


def _prep_inputs(x, T):
    bf16 = ml_dtypes.bfloat16
    # permute W columns to (q, kh, o%64, k%2) so every per-group lhsT slice of
    # the GEMM is contiguous (walrus: weights AP must have one free dim)
    Wp = (
        np.asarray(T, np.float32)
        .reshape(F, 2, 64, 4, 2)
        .transpose(0, 1, 3, 2, 4)
        .reshape(F, O * K)
    )
    fp8 = ml_dtypes.float8_e4m3
    W = np.ascontiguousarray(Wp).astype(fp8)
    xT = np.ascontiguousarray(x.T.astype(np.float32)).astype(fp8)
    mask = np.zeros((128, 64), dtype=bf16)
    mask[np.arange(128), np.arange(128) // 2] = 1.0
    dmask = np.zeros((128, 128), dtype=bf16)
    p = np.arange(128)
    dmask[:, :] = (p[:, None] // 2 == p[None, :] // 2).astype(bf16)
    in_maps = []
    for b in range(NCORES):
        in_maps.append(
            {
                "w": W,
                "mask": mask,
                "dmask": dmask,
                "xt": np.ascontiguousarray(np.roll(xT, -IB * b, axis=1)),
            }
        )
    return in_maps


def _assemble(x, results):
    c_full = np.concatenate([r["c"].T for r in results], axis=0)  # (B, O)
    return np.concatenate(
        [np.asarray(x, np.float32), c_full.astype(np.float32)], axis=1
    )


def _get_nc():
    if "nc" not in _cache:
        _cache["nc"] = _build()
    return _cache["nc"]


def kernel(x, T):
    from concourse.bass_utils import run_bass_kernel_spmd

    x = np.asarray(x)
    T = np.asarray(T)
    nc = _get_nc()
    res = run_bass_kernel_spmd(nc, _prep_inputs(x, T), list(range(NCORES)))
    return _assemble(x, res.results)


def run_traced(x, T, **kwargs):
    """Like kernel() but returns (output, BassKernelResults) with tracing on."""
    from concourse.bass_utils import run_bass_kernel_spmd

    x = np.asarray(x)
    T = np.asarray(T)
    nc = _get_nc()
    res = run_bass_kernel_spmd(
        nc, _prep_inputs(x, T), list(range(NCORES)), trace=True, **kwargs
    )
    return _assemble(x, res.results), res


# revision 16
# speedup vs baseline: 1.0461x; 1.0232x over previous
"""Trainium2 Bass kernel for nn_MinibatchDiscrimination (B=256, F=1024, O=128, K=8).

Computes out = concat([x, c], axis=1) where
    M = (x @ T.reshape(F, O*K)).reshape(B, O, K)
    c[i, o] = sum_{j != i} exp(-sum_k |M[j,o,k] - M[i,o,k]|)
(the reference's `sum_j exp(-d) - 1` equals the self-term-excluded sum since
 d_ii == 0 exactly).

Distribution: batch rows of c are sharded across 8 cores (32 rows each).
Every core redundantly computes the full GEMM (it needs all of M for the
pairwise reduction anyway; the GEMM is ~7% of the work).  SPMD divergence is
achieved purely through data: core b receives x^T with its batch columns
rotated so that its 32 owned rows sit at local columns [0, 32) — the pairwise
sum over j is permutation-invariant, so compile-time index offsets work for
every core.

Per-core device pipeline (all layouts use partition p = (o%64)*2 + (k%2),
column group g = 4*(o//64) + ((k//2)%4), i.e. 2 o-halves x 4 k-quarters;
matmul outputs may only be placed at partition offsets {0, 64}):
  1. DMA  w (bf16 F x O*K), xt (bf16 F x B, rolled), mask (bf16 128x32).
  2. GEMM Mt[p, g*B + j] = M[j, o(p,g), k(p,g)]   (TensorE, bf16, f32 PSUM),
     copied to SBUF as bf16 (full) + f32 (first 32 columns, tensor_scalar
     operands must be f32).
  2b. S[o, j] = sum_k M[j,o,k] via a mask-matmul over Mt; stored as
     -S/2 (f32, per (o,j)) and -S[:, :32] (f32 bias columns).
  3. For each local row i in [0,32), using |d| = 2*relu(d) - d and
     sum_k d_k = S_j - S_i:
     a. VectorE tensor_scalar:  R = relu(Mt - Mt[:, i])  (fused
        subtract+max-0, bf16, 4x mode), 8 column groups.
     b. ScalarE prefills the PSUM tile with -S_j/2, then TensorE mask-matmuls
        accumulate (start=False): h[o, j] = sum_k relu(d_k) - S[o,j]/2.
     c. ScalarE activation Exp(scale=-2, bias=-S[:, i]) with accum_out ->
        c_acc[:, i] = sum_j exp(-2h - S_i) = sum_j exp(-diffs);
        the self term e[:, i] is extracted to c_diag[:, i] (this also keeps
        the diagonal cancellation exact).
  4. c = c_acc - c_diag (VectorE), DMA out as (O, 32) f32.
Host gathers: c_full rows [32b, 32b+32) = core_b_out.T; output = [x | c_full].
"""

import numpy as np
import ml_dtypes

B, F, O, K = 256, 1024, 128, 8
NCORES = 8
IB = B // NCORES  # c rows owned per core

_cache = {}


def _build():
    from contextlib import ExitStack
    import concourse.bacc as bacc
    import concourse.tile as tile
    import concourse.mybir as mybir

    dt = mybir.dt
    Alu = mybir.AluOpType
    Act = mybir.ActivationFunctionType

    nc = bacc.Bacc(
        "TRN2", target_bir_lowering=False, debug=False, enable_asserts=False
    )
    w = nc.dram_tensor("w", (F, O * K), dt.float8e4, kind="ExternalInput").ap()
    xt = nc.dram_tensor("xt", (F, B), dt.float8e4, kind="ExternalInput").ap()
    mask = nc.dram_tensor("mask", (128, 64), dt.bfloat16, kind="ExternalInput").ap()
    dmask = nc.dram_tensor("dmask", (128, 128), dt.bfloat16, kind="ExternalInput").ap()
    cout = nc.dram_tensor("c", (O, IB), dt.float32, kind="ExternalOutput").ap()

    FC = F // 128  # contraction chunks
    G = 8  # column groups (2 o-halves x 4 k-quarters)

    with ExitStack() as ctx:
        tc = ctx.enter_context(tile.TileContext(nc))
        inpool = ctx.enter_context(tc.tile_pool(name="inp", bufs=1))
        mpool = ctx.enter_context(tc.tile_pool(name="mt", bufs=1))
        dpool = ctx.enter_context(tc.tile_pool(name="d", bufs=3))
        cpool = ctx.enter_context(tc.tile_pool(name="c", bufs=1))

        # single DMA per input: dma_start issue costs ~0.6us each on the
        # sequencer, so chunked loads gate the GEMM on issue rate, not BW
        x_sb = inpool.tile([128, FC * B], dt.float8e4, tag="xsb")
        nc.sync.dma_start(
            x_sb[:].rearrange("p (c j) -> p c j", c=FC),
            xt.rearrange("(c p) j -> p c j", p=128),
        )
        w_sb = inpool.tile([128, FC * O * K], dt.float8e4, tag="wsb")
        nc.sync.dma_start(
            w_sb[:].rearrange("p (c n) -> p c n", c=FC),
            w.rearrange("(c p) n -> p c n", p=128),
        )
        mask_sb = inpool.tile([128, 64], dt.bfloat16, tag="mask")
        nc.sync.dma_start(mask_sb[:], mask)
        dmask_sb = inpool.tile([128, 128], dt.bfloat16, tag="dmask")
        nc.sync.dma_start(dmask_sb[:], dmask)
        x_tiles = [x_sb[:, fc * B : (fc + 1) * B] for fc in range(FC)]
        w_tiles = [x_sb and w_sb[:, fc * O * K : (fc + 1) * O * K] for fc in range(FC)]

        # Mt: partition p=(o%64)*2+k%2, free j; one tile per group g = 4*(o//64)+k//2
        mt = [
            mpool.tile([128, B], dt.bfloat16, tag=f"mt{g}", name=f"mt{g}")
            for g in range(G)
        ]
        # f32 copies of the first IB columns of each group (tensor_scalar scalars)
        mtf = [
            mpool.tile([128, IB], dt.float32, tag=f"mtf{g}", name=f"mtf{g}")
            for g in range(G)
        ]
        # negated f32 columns (activation-Abs bias for the ScalarE row variant)
        nmtf = [
            mpool.tile([128, IB], dt.float32, tag=f"nmtf{g}", name=f"nmtf{g}")
            for g in range(G)
        ]

        with tc.tile_pool(name="gps", bufs=2, space="PSUM") as gps:
            # host pre-permuted w: col = (g*64 + o%64)*2 + k%2, g=4*(o//64)+k//2
            # g-outer so mt[0] (and the pairwise loop) starts as soon as the
            # last w/x chunk lands rather than after the whole GEMM
            for g in range(G):
                gm = gps.tile([128, B], dt.float32, tag="gm", name=f"gm{g}")
                for fc in range(FC):
                    nc.tensor.matmul(
                        gm[:],
                        w_tiles[fc][:, g * 128 : (g + 1) * 128],
                        x_tiles[fc][:],
                        start=(fc == 0),
                        stop=(fc == FC - 1),
                    )
                nc.scalar.copy(mt[g][:], gm[:])
                nc.scalar.copy(mtf[g][:], mt[g][:, 0:IB])
                nc.scalar.mul(nmtf[g][:], mt[g][:, 0:IB], -1.0)

        # S[o, j] = sum_k M[j, o, k].  Two forms are kept:
        #  - sneg[q]: -S/4 in the duplicated (o%64)*2+k%2 partition layout
        #    (bf16): streamed as the first pairwise matmul of each row with
        #    start=True, so the mask-sum over partition pairs seeds the PSUM
        #    with -S_j/2.
        #  - neg_s_bias: -bf16(S) as f32 in o-layout: the per-partition exp
        #    bias.  bf16(S/4)*4 == bf16(S) exactly, so the diagonal argument
        #    cancels to exactly 0 for every row and e[:, i] is one constant
        #    column, extracted once (c_diag0).
        sneg = [
            mpool.tile([128, B], dt.bfloat16, tag=f"sneg{q}", name=f"sneg{q}")
            for q in range(2)
        ]
        s_bf = mpool.tile([128, IB], dt.bfloat16, tag="sbf")
        neg_s_bias = mpool.tile([128, IB], dt.float32, tag="negsb")
        with tc.tile_pool(name="sps", bufs=1, space="PSUM") as spsp:
            s_ps = spsp.tile([128, B], dt.float32, tag="sps")
            for q in range(2):
                for kh in range(4):
                    g = 4 * q + kh
                    nc.tensor.matmul(
                        s_ps[64 * q : 64 * (q + 1), :],
                        mask_sb[:],
                        mt[g][:],
                        start=(kh == 0),
                        stop=(kh == 3),
                    )
            nc.scalar.copy(s_bf[:], s_ps[:, 0:IB])
            nc.scalar.mul(neg_s_bias[:], s_bf[:], -1.0)
            sd_ps = spsp.tile([128, B], dt.float32, tag="sdps")
            for q in range(2):
                for kh in range(4):
                    g = 4 * q + kh
                    nc.tensor.matmul(
                        sd_ps[:],
                        dmask_sb[:],
                        mt[g][:],
                        start=(kh == 0),
                        stop=(kh == 3),
                    )
                nc.scalar.mul(sneg[q][:], sd_ps[:], -0.25)

        c_acc = cpool.tile([128, IB], dt.float32, tag="cacc")
        c_diag = cpool.tile([128, IB], dt.float32, tag="cdiag")
        c_fin = cpool.tile([128, IB], dt.float32, tag="cfin")

        with tc.tile_pool(name="dps", bufs=2, space="PSUM") as dps, tc.tile_pool(
            name="eps", bufs=2, space="PSUM"
        ) as eps:
            first_mm = True
            for i in range(IB):
                act_row = i % 8 == 7  # rows computed via ScalarE Abs (balance)
                d = dpool.tile([128, G * B], dt.bfloat16, tag="d")
                for g in range(G):
                    if act_row:
                        nc.scalar.activation(
                            d[:, g * B : (g + 1) * B],
                            mt[g][:],
                            Act.Abs,
                            bias=nmtf[g][:, i : i + 1],
                        )
                    else:
                        nc.vector.tensor_scalar(
                            d[:, g * B : (g + 1) * B],
                            mt[g][:],
                            mtf[g][:, i : i + 1],
                            0.0,
                            Alu.subtract,
                            Alu.max,
                        )
                diffs = dps.tile([128, B], dt.float32, tag="diffs")
                for q in range(2):
                    mms = []
                    if not act_row:
                        # seed the PSUM quadrant with -S_j/2
                        mms.append(
                            nc.tensor.matmul(
                                diffs[64 * q : 64 * (q + 1), :],
                                mask_sb[:],
                                sneg[q][:],
                                start=True,
                                stop=False,
                                skip_group_check=True,
                            )
                        )
                    for kh in range(4):
                        g = 4 * q + kh
                        mms.append(
                            nc.tensor.matmul(
                                diffs[64 * q : 64 * (q + 1), :],
                                mask_sb[:],
                                d[:, g * B : (g + 1) * B],
                                start=(act_row and kh == 0),
                                stop=(kh == 3),
                                skip_group_check=True,
                            )
                        )
                    for mm in mms:
                        # every pairwise matmul reuses the identical mask
                        # weights: skip the LDWEIGHTS reload after the first
                        if not first_mm:
                            mm.ins.ldweights = False
                        first_mm = False
                e = eps.tile([128, B], dt.float32, tag="e")
                nc.scalar.activation(
                    e[:],
                    diffs[:],
                    Act.Exp,
                    scale=(-1.0 if act_row else -2.0),
                    bias=(0.0 if act_row else neg_s_bias[:, i : i + 1]),
                    accum_out=c_acc[:, i : i + 1],
                )
                if i == 0:
                    # diagonal argument is exactly 0 for every row (see S
                    # handling above), so one extracted column serves all
                    nc.scalar.copy(c_diag[:, 0:1], e[:, 0:1])

        nc.vector.tensor_scalar_sub(c_fin[:], c_acc[:], c_diag[:, 0:1])
        nc.sync.dma_start(cout, c_fin[:])

    nc.compile()
    return nc


def _prep_inputs(x, T):
    bf16 = ml_dtypes.bfloat16
    # permute W columns to (q, kh, o%64, k%2) so every per-group lhsT slice of
    # the GEMM is contiguous (walrus: weights AP must have one free dim)
    Wp = (
        np.asarray(T, np.float32)
        .reshape(F, 2, 64, 4, 2)
        .transpose(0, 1, 3, 2, 4)
        .reshape(F, O * K)
    )
    fp8 = ml_dtypes.float8_e4m3
    W = np.ascontiguousarray(Wp).astype(fp8)
    xT = np.ascontiguousarray(x.T.astype(np.float32)).astype(fp8)
    mask = np.zeros((128, 64), dtype=bf16)
    mask[np.arange(128), np.arange(128) // 2] = 1.0
    dmask = np.zeros((128, 128), dtype=bf16)
    p = np.arange(128)
    dmask[:, :] = (p[:, None] // 2 == p[None, :] // 2).astype(bf16)
    in_maps = []
    for b in range(NCORES):
        in_maps.append(
            {
                "w": W,
                "mask": mask,
                "dmask": dmask,
                "xt": np.ascontiguousarray(np.roll(xT, -IB * b, axis=1)),
            }
        )
    return in_maps


def _assemble(x, results):
    c_full = np.concatenate([r["c"].T for r in results], axis=0)  # (B, O)
    return np.concatenate(
        [np.asarray(x, np.float32), c_full.astype(np.float32)], axis=1
    )


def _get_nc():
    if "nc" not in _cache:
        _cache["nc"] = _build()
    return _cache["nc"]


def kernel(x, T):
    from concourse.bass_utils import run_bass_kernel_spmd

    x = np.asarray(x)
    T = np.asarray(T)
    nc = _get_nc()
    res = run_bass_kernel_spmd(nc, _prep_inputs(x, T), list(range(NCORES)))
    return _assemble(x, res.results)


def run_traced(x, T, **kwargs):
    """Like kernel() but returns (output, BassKernelResults) with tracing on."""
    from concourse.bass_utils import run_bass_kernel_spmd

    x = np.asarray(x)
    T = np.asarray(T)
    nc = _get_nc()
    res = run_bass_kernel_spmd(
        nc, _prep_inputs(x, T), list(range(NCORES)), trace=True, **kwargs
    )
    return _assemble(x, res.results), res


# revision 17
# speedup vs baseline: 1.0697x; 1.0226x over previous
"""Trainium2 Bass kernel for nn_MinibatchDiscrimination (B=256, F=1024, O=128, K=8).

Computes out = concat([x, c], axis=1) where
    M = (x @ T.reshape(F, O*K)).reshape(B, O, K)
    c[i, o] = sum_{j != i} exp(-sum_k |M[j,o,k] - M[i,o,k]|)
(the reference's `sum_j exp(-d) - 1` equals the self-term-excluded sum since
 d_ii == 0 exactly).

Distribution: batch rows of c are sharded across 8 cores (32 rows each).
Every core redundantly computes the full GEMM (it needs all of M for the
pairwise reduction anyway; the GEMM is ~7% of the work).  SPMD divergence is
achieved purely through data: core b receives x^T with its batch columns
rotated so that its 32 owned rows sit at local columns [0, 32) — the pairwise
sum over j is permutation-invariant, so compile-time index offsets work for
every core.

Per-core device pipeline (all layouts use partition p = (o%64)*2 + (k%2),
column group g = 4*(o//64) + ((k//2)%4), i.e. 2 o-halves x 4 k-quarters;
matmul outputs may only be placed at partition offsets {0, 64}):
  1. DMA  w (bf16 F x O*K), xt (bf16 F x B, rolled), mask (bf16 128x32).
  2. GEMM Mt[p, g*B + j] = M[j, o(p,g), k(p,g)]   (TensorE, bf16, f32 PSUM),
     copied to SBUF as bf16 (full) + f32 (first 32 columns, tensor_scalar
     operands must be f32).
  2b. S[o, j] = sum_k M[j,o,k] via a mask-matmul over Mt; stored as
     -S/2 (f32, per (o,j)) and -S[:, :32] (f32 bias columns).
  3. For each local row i in [0,32), using |d| = 2*relu(d) - d and
     sum_k d_k = S_j - S_i:
     a. VectorE tensor_scalar:  R = relu(Mt - Mt[:, i])  (fused
        subtract+max-0, bf16, 4x mode), 8 column groups.
     b. ScalarE prefills the PSUM tile with -S_j/2, then TensorE mask-matmuls
        accumulate (start=False): h[o, j] = sum_k relu(d_k) - S[o,j]/2.
     c. ScalarE activation Exp(scale=-2, bias=-S[:, i]) with accum_out ->
        c_acc[:, i] = sum_j exp(-2h - S_i) = sum_j exp(-diffs);
        the self term e[:, i] is extracted to c_diag[:, i] (this also keeps
        the diagonal cancellation exact).
  4. c = c_acc - c_diag (VectorE), DMA out as (O, 32) f32.
Host gathers: c_full rows [32b, 32b+32) = core_b_out.T; output = [x | c_full].
"""

import numpy as np
import ml_dtypes

B, F, O, K = 256, 1024, 128, 8
NCORES = 8
IB = B // NCORES  # c rows owned per core

_cache = {}


def _build():
    from contextlib import ExitStack
    import concourse.bacc as bacc
    import concourse.tile as tile
    import concourse.mybir as mybir

    dt = mybir.dt
    Alu = mybir.AluOpType
    Act = mybir.ActivationFunctionType

    nc = bacc.Bacc(
        "TRN2", target_bir_lowering=False, debug=False, enable_asserts=False
    )
    w = nc.dram_tensor("w", (F, O * K), dt.float8e4, kind="ExternalInput").ap()
    xt = nc.dram_tensor("xt", (F, B), dt.float8e4, kind="ExternalInput").ap()
    mask = nc.dram_tensor("mask", (128, 64), dt.bfloat16, kind="ExternalInput").ap()
    dmask = nc.dram_tensor("dmask", (128, 128), dt.bfloat16, kind="ExternalInput").ap()
    cout = nc.dram_tensor("c", (O, IB), dt.float32, kind="ExternalOutput").ap()

    FC = F // 128  # contraction chunks
    G = 8  # column groups (2 o-halves x 4 k-quarters)

    with ExitStack() as ctx:
        tc = ctx.enter_context(tile.TileContext(nc))
        inpool = ctx.enter_context(tc.tile_pool(name="inp", bufs=1))
        mpool = ctx.enter_context(tc.tile_pool(name="mt", bufs=1))
        dpool = ctx.enter_context(tc.tile_pool(name="d", bufs=3))
        cpool = ctx.enter_context(tc.tile_pool(name="c", bufs=1))

        # single DMA per input: dma_start issue costs ~0.6us each on the
        # sequencer, so chunked loads gate the GEMM on issue rate, not BW
        w_sb = inpool.tile([128, FC * O * K], dt.float8e4, tag="wsb")
        nc.sync.dma_start(
            w_sb[:].rearrange("p (c n) -> p c n", c=FC),
            w.rearrange("(c p) n -> p c n", p=128),
        )
        x_sb = inpool.tile([128, FC * B], dt.float8e4, tag="xsb")
        nc.gpsimd.dma_start(
            x_sb[:].rearrange("p (c j) -> p c j", c=FC),
            xt.rearrange("(c p) j -> p c j", p=128),
        )
        mask_sb = inpool.tile([128, 64], dt.bfloat16, tag="mask")
        nc.gpsimd.dma_start(mask_sb[:], mask)
        dmask_sb = inpool.tile([128, 128], dt.bfloat16, tag="dmask")
        nc.gpsimd.dma_start(dmask_sb[:], dmask)
        x_tiles = [x_sb[:, fc * B : (fc + 1) * B] for fc in range(FC)]
        w_tiles = [x_sb and w_sb[:, fc * O * K : (fc + 1) * O * K] for fc in range(FC)]

        # Mt: partition p=(o%64)*2+k%2, free j; one tile per group g = 4*(o//64)+k//2
        mt = [
            mpool.tile([128, B], dt.bfloat16, tag=f"mt{g}", name=f"mt{g}")
            for g in range(G)
        ]
        # f32 copies of the first IB columns of each group (tensor_scalar scalars)
        mtf = [
            mpool.tile([128, IB], dt.float32, tag=f"mtf{g}", name=f"mtf{g}")
            for g in range(G)
        ]
        # negated f32 columns (activation-Abs bias for the ScalarE row variant)
        nmtf = [
            mpool.tile([128, IB], dt.float32, tag=f"nmtf{g}", name=f"nmtf{g}")
            for g in range(G)
        ]

        with tc.tile_pool(name="gps", bufs=2, space="PSUM") as gps:
            # host pre-permuted w: col = (g*64 + o%64)*2 + k%2, g=4*(o//64)+k//2
            # g-outer so mt[0] (and the pairwise loop) starts as soon as the
            # last w/x chunk lands rather than after the whole GEMM
            for g in range(G):
                gm = gps.tile([128, B], dt.float32, tag="gm", name=f"gm{g}")
                for fc in range(FC):
                    nc.tensor.matmul(
                        gm[:],
                        w_tiles[fc][:, g * 128 : (g + 1) * 128],
                        x_tiles[fc][:],
                        start=(fc == 0),
                        stop=(fc == FC - 1),
                    )
                nc.scalar.copy(mt[g][:], gm[:])
                nc.scalar.copy(mtf[g][:], mt[g][:, 0:IB])
                nc.scalar.mul(nmtf[g][:], mt[g][:, 0:IB], -1.0)

        # S[o, j] = sum_k M[j, o, k].  Two forms are kept:
        #  - sneg[q]: -S/4 in the duplicated (o%64)*2+k%2 partition layout
        #    (bf16): streamed as the first pairwise matmul of each row with
        #    start=True, so the mask-sum over partition pairs seeds the PSUM
        #    with -S_j/2.
        #  - neg_s_bias: -bf16(S) as f32 in o-layout: the per-partition exp
        #    bias.  bf16(S/4)*4 == bf16(S) exactly, so the diagonal argument
        #    cancels to exactly 0 for every row and e[:, i] is one constant
        #    column, extracted once (c_diag0).
        sneg = [
            mpool.tile([128, B], dt.bfloat16, tag=f"sneg{q}", name=f"sneg{q}")
            for q in range(2)
        ]
        s_bf = mpool.tile([128, IB], dt.bfloat16, tag="sbf")
        neg_s_bias = mpool.tile([128, IB], dt.float32, tag="negsb")
        with tc.tile_pool(name="sps", bufs=1, space="PSUM") as spsp:
            s_ps = spsp.tile([128, B], dt.float32, tag="sps")
            for q in range(2):
                for kh in range(4):
                    g = 4 * q + kh
                    nc.tensor.matmul(
                        s_ps[64 * q : 64 * (q + 1), :],
                        mask_sb[:],
                        mt[g][:],
                        start=(kh == 0),
                        stop=(kh == 3),
                    )
            nc.scalar.copy(s_bf[:], s_ps[:, 0:IB])
            nc.scalar.mul(neg_s_bias[:], s_bf[:], -1.0)
            sd_ps = spsp.tile([128, B], dt.float32, tag="sdps")
            for q in range(2):
                for kh in range(4):
                    g = 4 * q + kh
                    nc.tensor.matmul(
                        sd_ps[:],
                        dmask_sb[:],
                        mt[g][:],
                        start=(kh == 0),
                        stop=(kh == 3),
                    )
                nc.scalar.mul(sneg[q][:], sd_ps[:], -0.25)

        c_acc = cpool.tile([128, IB], dt.float32, tag="cacc")
        c_diag = cpool.tile([128, IB], dt.float32, tag="cdiag")
        c_fin = cpool.tile([128, IB], dt.float32, tag="cfin")

        with tc.tile_pool(name="dps", bufs=2, space="PSUM") as dps, tc.tile_pool(
            name="eps", bufs=2, space="PSUM"
        ) as eps:
            first_mm = True
            for i in range(IB):
                act_row = i % 8 == 7  # rows computed via ScalarE Abs (balance)
                d = dpool.tile([128, G * B], dt.bfloat16, tag="d")
                for g in range(G):
                    if act_row:
                        nc.scalar.activation(
                            d[:, g * B : (g + 1) * B],
                            mt[g][:],
                            Act.Abs,
                            bias=nmtf[g][:, i : i + 1],
                        )
                    else:
                        nc.vector.tensor_scalar(
                            d[:, g * B : (g + 1) * B],
                            mt[g][:],
                            mtf[g][:, i : i + 1],
                            0.0,
                            Alu.subtract,
                            Alu.max,
                        )
                diffs = dps.tile([128, B], dt.float32, tag="diffs")
                for q in range(2):
                    mms = []
                    if not act_row:
                        # seed the PSUM quadrant with -S_j/2
                        mms.append(
                            nc.tensor.matmul(
                                diffs[64 * q : 64 * (q + 1), :],
                                mask_sb[:],
                                sneg[q][:],
                                start=True,
                                stop=False,
                                skip_group_check=True,
                            )
                        )
                    for kh in range(4):
                        g = 4 * q + kh
                        mms.append(
                            nc.tensor.matmul(
                                diffs[64 * q : 64 * (q + 1), :],
                                mask_sb[:],
                                d[:, g * B : (g + 1) * B],
                                start=(act_row and kh == 0),
                                stop=(kh == 3),
                                skip_group_check=True,
                            )
                        )
                    for mm in mms:
                        # every pairwise matmul reuses the identical mask
                        # weights: skip the LDWEIGHTS reload after the first
                        if not first_mm:
                            mm.ins.ldweights = False
                        first_mm = False
                e = eps.tile([128, B], dt.float32, tag="e")
                nc.scalar.activation(
                    e[:],
                    diffs[:],
                    Act.Exp,
                    scale=(-1.0 if act_row else -2.0),
                    bias=(0.0 if act_row else neg_s_bias[:, i : i + 1]),
                    accum_out=c_acc[:, i : i + 1],
                )
                if i == 0:
                    # diagonal argument is exactly 0 for every row (see S
                    # handling above), so one extracted column serves all
                    nc.scalar.copy(c_diag[:, 0:1], e[:, 0:1])

        nc.vector.tensor_scalar_sub(c_fin[:], c_acc[:], c_diag[:, 0:1])
        nc.sync.dma_start(cout, c_fin[:])

    nc.compile()
    return nc


def _prep_inputs(x, T):
    bf16 = ml_dtypes.bfloat16
    # permute W columns to (q, kh, o%64, k%2) so every per-group lhsT slice of
    # the GEMM is contiguous (walrus: weights AP must have one free dim)
    Wp = (
        np.asarray(T, np.float32)
        .reshape(F, 2, 64, 4, 2)
        .transpose(0, 1, 3, 2, 4)
        .reshape(F, O * K)
    )
    fp8 = ml_dtypes.float8_e4m3
    W = np.ascontiguousarray(Wp).astype(fp8)
    xT = np.ascontiguousarray(x.T.astype(np.float32)).astype(fp8)
    mask = np.zeros((128, 64), dtype=bf16)
    mask[np.arange(128), np.arange(128) // 2] = 1.0
    dmask = np.zeros((128, 128), dtype=bf16)
    p = np.arange(128)
    dmask[:, :] = (p[:, None] // 2 == p[None, :] // 2).astype(bf16)
    in_maps = []
    for b in range(NCORES):
        in_maps.append(
            {
                "w": W,
                "mask": mask,
                "dmask": dmask,
                "xt": np.ascontiguousarray(np.roll(xT, -IB * b, axis=1)),
            }
        )
    return in_maps


def _assemble(x, results):
    c_full = np.concatenate([r["c"].T for r in results], axis=0)  # (B, O)
    return np.concatenate(
        [np.asarray(x, np.float32), c_full.astype(np.float32)], axis=1
    )


def _get_nc():
    if "nc" not in _cache:
        _cache["nc"] = _build()
    return _cache["nc"]


def kernel(x, T):
    from concourse.bass_utils import run_bass_kernel_spmd

    x = np.asarray(x)
    T = np.asarray(T)
    nc = _get_nc()
    res = run_bass_kernel_spmd(nc, _prep_inputs(x, T), list(range(NCORES)))
    return _assemble(x, res.results)


def run_traced(x, T, **kwargs):
    """Like kernel() but returns (output, BassKernelResults) with tracing on."""
    from concourse.bass_utils import run_bass_kernel_spmd

    x = np.asarray(x)
    T = np.asarray(T)
    nc = _get_nc()
    res = run_bass_kernel_spmd(
        nc, _prep_inputs(x, T), list(range(NCORES)), trace=True, **kwargs
    )
    return _assemble(x, res.results), res


# revision 18
# speedup vs baseline: 1.0878x; 1.0169x over previous
"""Trainium2 Bass kernel for nn_MinibatchDiscrimination (B=256, F=1024, O=128, K=8).

Computes out = concat([x, c], axis=1) where
    M = (x @ T.reshape(F, O*K)).reshape(B, O, K)
    c[i, o] = sum_{j != i} exp(-sum_k |M[j,o,k] - M[i,o,k]|)
(the reference's `sum_j exp(-d) - 1` equals the self-term-excluded sum since
 d_ii == 0 exactly).

Distribution: batch rows of c are sharded across 8 cores (32 rows each).
Every core redundantly computes the full GEMM (it needs all of M for the
pairwise reduction anyway; the GEMM is ~7% of the work).  SPMD divergence is
achieved purely through data: core b receives x^T with its batch columns
rotated so that its 32 owned rows sit at local columns [0, 32) — the pairwise
sum over j is permutation-invariant, so compile-time index offsets work for
every core.

Per-core device pipeline (all layouts use partition p = (o%64)*2 + (k%2),
column group g = 4*(o//64) + ((k//2)%4), i.e. 2 o-halves x 4 k-quarters;
matmul outputs may only be placed at partition offsets {0, 64}):
  1. DMA  w (bf16 F x O*K), xt (bf16 F x B, rolled), mask (bf16 128x32).
  2. GEMM Mt[p, g*B + j] = M[j, o(p,g), k(p,g)]   (TensorE, bf16, f32 PSUM),
     copied to SBUF as bf16 (full) + f32 (first 32 columns, tensor_scalar
     operands must be f32).
  2b. S[o, j] = sum_k M[j,o,k] via a mask-matmul over Mt; stored as
     -S/2 (f32, per (o,j)) and -S[:, :32] (f32 bias columns).
  3. For each local row i in [0,32), using |d| = 2*relu(d) - d and
     sum_k d_k = S_j - S_i:
     a. VectorE tensor_scalar:  R = relu(Mt - Mt[:, i])  (fused
        subtract+max-0, bf16, 4x mode), 8 column groups.
     b. ScalarE prefills the PSUM tile with -S_j/2, then TensorE mask-matmuls
        accumulate (start=False): h[o, j] = sum_k relu(d_k) - S[o,j]/2.
     c. ScalarE activation Exp(scale=-2, bias=-S[:, i]) with accum_out ->
        c_acc[:, i] = sum_j exp(-2h - S_i) = sum_j exp(-diffs);
        the self term e[:, i] is extracted to c_diag[:, i] (this also keeps
        the diagonal cancellation exact).
  4. c = c_acc - c_diag (VectorE), DMA out as (O, 32) f32.
Host gathers: c_full rows [32b, 32b+32) = core_b_out.T; output = [x | c_full].
"""

import numpy as np
import ml_dtypes

B, F, O, K = 256, 1024, 128, 8
NCORES = 8
IB = B // NCORES  # c rows owned per core

_cache = {}


def _build():
    from contextlib import ExitStack
    import concourse.bacc as bacc
    import concourse.tile as tile
    import concourse.mybir as mybir

    dt = mybir.dt
    Alu = mybir.AluOpType
    Act = mybir.ActivationFunctionType

    nc = bacc.Bacc(
        "TRN2", target_bir_lowering=False, debug=False, enable_asserts=False
    )
    w = nc.dram_tensor("w", (128, F // 128 * O * K), dt.float8e4, kind="ExternalInput").ap()
    xt = nc.dram_tensor("xt", (128, F // 128 * B), dt.float8e4, kind="ExternalInput").ap()
    mask = nc.dram_tensor("mask", (128, 64), dt.bfloat16, kind="ExternalInput").ap()
    dmask = nc.dram_tensor("dmask", (128, 128), dt.bfloat16, kind="ExternalInput").ap()
    cout = nc.dram_tensor("c", (O, IB), dt.float32, kind="ExternalOutput").ap()

    FC = F // 128  # contraction chunks
    G = 8  # column groups (2 o-halves x 4 k-quarters)

    with ExitStack() as ctx:
        tc = ctx.enter_context(tile.TileContext(nc))
        inpool = ctx.enter_context(tc.tile_pool(name="inp", bufs=1))
        mpool = ctx.enter_context(tc.tile_pool(name="mt", bufs=1))
        dpool = ctx.enter_context(tc.tile_pool(name="d", bufs=3))
        cpool = ctx.enter_context(tc.tile_pool(name="c", bufs=1))

        # single DMA per input: dma_start issue costs ~0.6us each on the
        # sequencer, so chunked loads gate the GEMM on issue rate, not BW
        # w/xt are shipped pre-interleaved as the exact SBUF image, so these
        # DMAs are fully contiguous (strided loads run at ~half DMA BW)
        w_sb = inpool.tile([128, FC * O * K], dt.float8e4, tag="wsb")
        nc.sync.dma_start(w_sb[:], w)
        x_sb = inpool.tile([128, FC * B], dt.float8e4, tag="xsb")
        nc.gpsimd.dma_start(x_sb[:], xt)
        mask_sb = inpool.tile([128, 64], dt.bfloat16, tag="mask")
        nc.gpsimd.dma_start(mask_sb[:], mask)
        dmask_sb = inpool.tile([128, 128], dt.bfloat16, tag="dmask")
        nc.gpsimd.dma_start(dmask_sb[:], dmask)
        x_tiles = [x_sb[:, fc * B : (fc + 1) * B] for fc in range(FC)]
        w_tiles = [x_sb and w_sb[:, fc * O * K : (fc + 1) * O * K] for fc in range(FC)]

        # Mt: partition p=(o%64)*2+k%2, free j; one tile per group g = 4*(o//64)+k//2
        mt = [
            mpool.tile([128, B], dt.bfloat16, tag=f"mt{g}", name=f"mt{g}")
            for g in range(G)
        ]
        # f32 copies of the first IB columns of each group (tensor_scalar scalars)
        mtf = [
            mpool.tile([128, IB], dt.float32, tag=f"mtf{g}", name=f"mtf{g}")
            for g in range(G)
        ]
        # negated f32 columns (activation-Abs bias for the ScalarE row variant)
        nmtf = [
            mpool.tile([128, IB], dt.float32, tag=f"nmtf{g}", name=f"nmtf{g}")
            for g in range(G)
        ]

        with tc.tile_pool(name="gps", bufs=2, space="PSUM") as gps:
            # host pre-permuted w: col = (g*64 + o%64)*2 + k%2, g=4*(o//64)+k//2
            # g-outer so mt[0] (and the pairwise loop) starts as soon as the
            # last w/x chunk lands rather than after the whole GEMM
            for g in range(G):
                gm = gps.tile([128, B], dt.float32, tag="gm", name=f"gm{g}")
                for fc in range(FC):
                    nc.tensor.matmul(
                        gm[:],
                        w_tiles[fc][:, g * 128 : (g + 1) * 128],
                        x_tiles[fc][:],
                        start=(fc == 0),
                        stop=(fc == FC - 1),
                    )
                nc.scalar.copy(mt[g][:], gm[:])
                nc.scalar.copy(mtf[g][:], mt[g][:, 0:IB])
                nc.scalar.mul(nmtf[g][:], mt[g][:, 0:IB], -1.0)

        # S[o, j] = sum_k M[j, o, k].  Two forms are kept:
        #  - sneg[q]: -S/4 in the duplicated (o%64)*2+k%2 partition layout
        #    (bf16): streamed as the first pairwise matmul of each row with
        #    start=True, so the mask-sum over partition pairs seeds the PSUM
        #    with -S_j/2.
        #  - neg_s_bias: -bf16(S) as f32 in o-layout: the per-partition exp
        #    bias.  bf16(S/4)*4 == bf16(S) exactly, so the diagonal argument
        #    cancels to exactly 0 for every row and e[:, i] is one constant
        #    column, extracted once (c_diag0).
        sneg = [
            mpool.tile([128, B], dt.bfloat16, tag=f"sneg{q}", name=f"sneg{q}")
            for q in range(2)
        ]
        s_bf = mpool.tile([128, IB], dt.bfloat16, tag="sbf")
        neg_s_bias = mpool.tile([128, IB], dt.float32, tag="negsb")
        with tc.tile_pool(name="sps", bufs=1, space="PSUM") as spsp:
            s_ps = spsp.tile([128, B], dt.float32, tag="sps")
            for q in range(2):
                for kh in range(4):
                    g = 4 * q + kh
                    nc.tensor.matmul(
                        s_ps[64 * q : 64 * (q + 1), :],
                        mask_sb[:],
                        mt[g][:],
                        start=(kh == 0),
                        stop=(kh == 3),
                    )
            nc.scalar.copy(s_bf[:], s_ps[:, 0:IB])
            nc.scalar.mul(neg_s_bias[:], s_bf[:], -1.0)
            sd_ps = spsp.tile([128, B], dt.float32, tag="sdps")
            for q in range(2):
                for kh in range(4):
                    g = 4 * q + kh
                    nc.tensor.matmul(
                        sd_ps[:],
                        dmask_sb[:],
                        mt[g][:],
                        start=(kh == 0),
                        stop=(kh == 3),
                    )
                nc.scalar.mul(sneg[q][:], sd_ps[:], -0.25)

        c_acc = cpool.tile([128, IB], dt.float32, tag="cacc")
        c_diag = cpool.tile([128, IB], dt.float32, tag="cdiag")
        c_fin = cpool.tile([128, IB], dt.float32, tag="cfin")

        with tc.tile_pool(name="dps", bufs=2, space="PSUM") as dps, tc.tile_pool(
            name="eps", bufs=2, space="PSUM"
        ) as eps:
            first_mm = True
            for i in range(IB):
                act_row = i % 8 == 7  # rows computed via ScalarE Abs (balance)
                d = dpool.tile([128, G * B], dt.bfloat16, tag="d")
                for g in range(G):
                    if act_row:
                        nc.scalar.activation(
                            d[:, g * B : (g + 1) * B],
                            mt[g][:],
                            Act.Abs,
                            bias=nmtf[g][:, i : i + 1],
                        )
                    else:
                        nc.vector.tensor_scalar(
                            d[:, g * B : (g + 1) * B],
                            mt[g][:],
                            mtf[g][:, i : i + 1],
                            0.0,
                            Alu.subtract,
                            Alu.max,
                        )
                diffs = dps.tile([128, B], dt.float32, tag="diffs")
                for q in range(2):
                    mms = []
                    if not act_row:
                        # seed the PSUM quadrant with -S_j/2
                        mms.append(
                            nc.tensor.matmul(
                                diffs[64 * q : 64 * (q + 1), :],
                                mask_sb[:],
                                sneg[q][:],
                                start=True,
                                stop=False,
                                skip_group_check=True,
                            )
                        )
                    for kh in range(4):
                        g = 4 * q + kh
                        mms.append(
                            nc.tensor.matmul(
                                diffs[64 * q : 64 * (q + 1), :],
                                mask_sb[:],
                                d[:, g * B : (g + 1) * B],
                                start=(act_row and kh == 0),
                                stop=(kh == 3),
                                skip_group_check=True,
                            )
                        )
                    for mm in mms:
                        # every pairwise matmul reuses the identical mask
                        # weights: skip the LDWEIGHTS reload after the first
                        if not first_mm:
                            mm.ins.ldweights = False
                        first_mm = False
                e = eps.tile([128, B], dt.float32, tag="e")
                nc.scalar.activation(
                    e[:],
                    diffs[:],
                    Act.Exp,
                    scale=(-1.0 if act_row else -2.0),
                    bias=(0.0 if act_row else neg_s_bias[:, i : i + 1]),
                    accum_out=c_acc[:, i : i + 1],
                )
                if i == 0:
                    # diagonal argument is exactly 0 for every row (see S
                    # handling above), so one extracted column serves all
                    nc.scalar.copy(c_diag[:, 0:1], e[:, 0:1])

        nc.vector.tensor_scalar_sub(c_fin[:], c_acc[:], c_diag[:, 0:1])
        nc.sync.dma_start(cout, c_fin[:])

    nc.compile()
    return nc


def _prep_inputs(x, T):
    bf16 = ml_dtypes.bfloat16
    # permute W columns to (q, kh, o%64, k%2) so every per-group lhsT slice of
    # the GEMM is contiguous (walrus: weights AP must have one free dim)
    Wp = (
        np.asarray(T, np.float32)
        .reshape(F, 2, 64, 4, 2)
        .transpose(0, 1, 3, 2, 4)
        .reshape(F, O * K)
    )
    fp8 = ml_dtypes.float8_e4m3
    # SBUF-image interleave: row p holds [chunk0 | chunk1 | ...] where
    # chunk c covers input features [128c, 128c+128)
    W = np.ascontiguousarray(
        Wp.reshape(F // 128, 128, O * K).transpose(1, 0, 2).reshape(128, -1)
    ).astype(fp8)
    xTf = x.T.astype(np.float32)
    mask = np.zeros((128, 64), dtype=bf16)
    mask[np.arange(128), np.arange(128) // 2] = 1.0
    dmask = np.zeros((128, 128), dtype=bf16)
    p = np.arange(128)
    dmask[:, :] = (p[:, None] // 2 == p[None, :] // 2).astype(bf16)
    in_maps = []
    for b in range(NCORES):
        in_maps.append(
            {
                "w": W,
                "mask": mask,
                "dmask": dmask,
                "xt": np.ascontiguousarray(
                    np.roll(xTf, -IB * b, axis=1)
                    .reshape(F // 128, 128, B)
                    .transpose(1, 0, 2)
                    .reshape(128, -1)
                ).astype(fp8),
            }
        )
    return in_maps


def _assemble(x, results):
    c_full = np.concatenate([r["c"].T for r in results], axis=0)  # (B, O)
    return np.concatenate(
        [np.asarray(x, np.float32), c_full.astype(np.float32)], axis=1
    )


def _get_nc():
    if "nc" not in _cache:
        _cache["nc"] = _build()
    return _cache["nc"]


def kernel(x, T):
    from concourse.bass_utils import run_bass_kernel_spmd

    x = np.asarray(x)
    T = np.asarray(T)
    nc = _get_nc()
    res = run_bass_kernel_spmd(nc, _prep_inputs(x, T), list(range(NCORES)))
    return _assemble(x, res.results)


def run_traced(x, T, **kwargs):
    """Like kernel() but returns (output, BassKernelResults) with tracing on."""
    from concourse.bass_utils import run_bass_kernel_spmd

    x = np.asarray(x)
    T = np.asarray(T)
    nc = _get_nc()
    res = run_bass_kernel_spmd(
        nc, _prep_inputs(x, T), list(range(NCORES)), trace=True, **kwargs
    )
    return _assemble(x, res.results), res


# revision 20
# speedup vs baseline: 1.1023x; 1.0133x over previous
"""Trainium2 Bass kernel for nn_MinibatchDiscrimination (B=256, F=1024, O=128, K=8).

Computes out = concat([x, c], axis=1) where
    M = (x @ T.reshape(F, O*K)).reshape(B, O, K)
    c[i, o] = sum_{j != i} exp(-sum_k |M[j,o,k] - M[i,o,k]|)
(the reference's `sum_j exp(-d) - 1` equals the self-term-excluded sum since
 d_ii == 0 exactly).

Distribution: batch rows of c are sharded across 8 cores (32 rows each).
Every core redundantly computes the full GEMM (it needs all of M for the
pairwise reduction anyway; the GEMM is ~7% of the work).  SPMD divergence is
achieved purely through data: core b receives x^T with its batch columns
rotated so that its 32 owned rows sit at local columns [0, 32) — the pairwise
sum over j is permutation-invariant, so compile-time index offsets work for
every core.

Per-core device pipeline (all layouts use partition p = (o%64)*2 + (k%2),
column group g = 4*(o//64) + ((k//2)%4), i.e. 2 o-halves x 4 k-quarters;
matmul outputs may only be placed at partition offsets {0, 64}):
  1. DMA  w (bf16 F x O*K), xt (bf16 F x B, rolled), mask (bf16 128x32).
  2. GEMM Mt[p, g*B + j] = M[j, o(p,g), k(p,g)]   (TensorE, bf16, f32 PSUM),
     copied to SBUF as bf16 (full) + f32 (first 32 columns, tensor_scalar
     operands must be f32).
  2b. S[o, j] = sum_k M[j,o,k] via a mask-matmul over Mt; stored as
     -S/2 (f32, per (o,j)) and -S[:, :32] (f32 bias columns).
  3. For each local row i in [0,32), using |d| = 2*relu(d) - d and
     sum_k d_k = S_j - S_i:
     a. VectorE tensor_scalar:  R = relu(Mt - Mt[:, i])  (fused
        subtract+max-0, bf16, 4x mode), 8 column groups.
     b. ScalarE prefills the PSUM tile with -S_j/2, then TensorE mask-matmuls
        accumulate (start=False): h[o, j] = sum_k relu(d_k) - S[o,j]/2.
     c. ScalarE activation Exp(scale=-2, bias=-S[:, i]) with accum_out ->
        c_acc[:, i] = sum_j exp(-2h - S_i) = sum_j exp(-diffs);
        the self term e[:, i] is extracted to c_diag[:, i] (this also keeps
        the diagonal cancellation exact).
  4. c = c_acc - c_diag (VectorE), DMA out as (O, 32) f32.
Host gathers: c_full rows [32b, 32b+32) = core_b_out.T; output = [x | c_full].
"""

import numpy as np
import ml_dtypes

B, F, O, K = 256, 1024, 128, 8
NCORES = 8
IB = B // NCORES  # c rows owned per core

_cache = {}


def _build():
    from contextlib import ExitStack
    import concourse.bacc as bacc
    import concourse.tile as tile
    import concourse.mybir as mybir

    dt = mybir.dt
    Alu = mybir.AluOpType
    Act = mybir.ActivationFunctionType

    nc = bacc.Bacc(
        "TRN2", target_bir_lowering=False, debug=False, enable_asserts=False
    )
    w = nc.dram_tensor("w", (128, F // 128 * O * K), dt.float8e4, kind="ExternalInput").ap()
    xt = nc.dram_tensor("xt", (128, F // 128 * B), dt.float8e4, kind="ExternalInput").ap()
    mask = nc.dram_tensor("mask", (128, 64), dt.bfloat16, kind="ExternalInput").ap()
    dmask = nc.dram_tensor("dmask", (128, 128), dt.bfloat16, kind="ExternalInput").ap()
    cout = nc.dram_tensor("c", (O, IB), dt.float32, kind="ExternalOutput").ap()
    cout2 = nc.dram_tensor("c2", (O, 160), dt.float32, kind="ExternalOutput").ap()

    FC = F // 128  # contraction chunks
    G = 8  # column groups (2 o-halves x 4 k-quarters)

    with ExitStack() as ctx:
        tc = ctx.enter_context(tile.TileContext(nc))
        inpool = ctx.enter_context(tc.tile_pool(name="inp", bufs=1))
        mpool = ctx.enter_context(tc.tile_pool(name="mt", bufs=1))
        dpool = ctx.enter_context(tc.tile_pool(name="d", bufs=3))
        cpool = ctx.enter_context(tc.tile_pool(name="c", bufs=1))

        # single DMA per input: dma_start issue costs ~0.6us each on the
        # sequencer, so chunked loads gate the GEMM on issue rate, not BW
        # w/xt are shipped pre-interleaved as the exact SBUF image, so these
        # DMAs are fully contiguous (strided loads run at ~half DMA BW)
        w_sb = inpool.tile([128, FC * O * K], dt.float8e4, tag="wsb")
        nc.sync.dma_start(w_sb[:], w)
        x_sb = inpool.tile([128, FC * B], dt.float8e4, tag="xsb")
        nc.gpsimd.dma_start(x_sb[:], xt)
        mask_sb = inpool.tile([128, 64], dt.bfloat16, tag="mask")
        nc.gpsimd.dma_start(mask_sb[:], mask)
        dmask_sb = inpool.tile([128, 128], dt.bfloat16, tag="dmask")
        nc.gpsimd.dma_start(dmask_sb[:], dmask)
        x_tiles = [x_sb[:, fc * B : (fc + 1) * B] for fc in range(FC)]
        w_tiles = [x_sb and w_sb[:, fc * O * K : (fc + 1) * O * K] for fc in range(FC)]

        # Mt: partition p=(o%64)*2+k%2, free j; one tile per group g = 4*(o//64)+k//2
        mt = [
            mpool.tile([128, B], dt.bfloat16, tag=f"mt{g}", name=f"mt{g}")
            for g in range(G)
        ]
        # f32 copies of the first IB columns of each group (tensor_scalar scalars)
        mtf = [
            mpool.tile([128, IB], dt.float32, tag=f"mtf{g}", name=f"mtf{g}")
            for g in range(G)
        ]
        # negated f32 columns (activation-Abs bias for the ScalarE row variant)
        nmtf = [
            mpool.tile([128, IB], dt.float32, tag=f"nmtf{g}", name=f"nmtf{g}")
            for g in range(G)
        ]

        with tc.tile_pool(name="gps", bufs=2, space="PSUM") as gps:
            # host pre-permuted w: col = (g*64 + o%64)*2 + k%2, g=4*(o//64)+k//2
            # g-outer so mt[0] (and the pairwise loop) starts as soon as the
            # last w/x chunk lands rather than after the whole GEMM
            for g in range(G):
                gm = gps.tile([128, B], dt.float32, tag="gm", name=f"gm{g}")
                for fc in range(FC):
                    nc.tensor.matmul(
                        gm[:],
                        w_tiles[fc][:, g * 128 : (g + 1) * 128],
                        x_tiles[fc][:],
                        start=(fc == 0),
                        stop=(fc == FC - 1),
                    )
                nc.scalar.copy(mt[g][:], gm[:])
                nc.scalar.copy(mtf[g][:], mt[g][:, 0:IB])
                nc.scalar.mul(nmtf[g][:], mt[g][:, 0:IB], -1.0)

        # S[o, j] = sum_k M[j, o, k].  Two forms are kept:
        #  - sneg[q]: -S/4 in the duplicated (o%64)*2+k%2 partition layout
        #    (bf16): streamed as the first pairwise matmul of each row with
        #    start=True, so the mask-sum over partition pairs seeds the PSUM
        #    with -S_j/2.
        #  - neg_s_bias: -bf16(S) as f32 in o-layout: the per-partition exp
        #    bias.  bf16(S/4)*4 == bf16(S) exactly, so the diagonal argument
        #    cancels to exactly 0 for every row and e[:, i] is one constant
        #    column, extracted once (c_diag0).
        sneg = [
            mpool.tile([128, B], dt.bfloat16, tag=f"sneg{q}", name=f"sneg{q}")
            for q in range(2)
        ]
        s_bf = mpool.tile([128, IB], dt.bfloat16, tag="sbf")
        neg_s_bias = mpool.tile([128, IB], dt.float32, tag="negsb")
        with tc.tile_pool(name="sps", bufs=1, space="PSUM") as spsp:
            s_ps = spsp.tile([128, B], dt.float32, tag="sps")
            for q in range(2):
                for kh in range(4):
                    g = 4 * q + kh
                    nc.tensor.matmul(
                        s_ps[64 * q : 64 * (q + 1), :],
                        mask_sb[:],
                        mt[g][:],
                        start=(kh == 0),
                        stop=(kh == 3),
                    )
            nc.scalar.copy(s_bf[:], s_ps[:, 0:IB])
            nc.scalar.mul(neg_s_bias[:], s_bf[:], -1.0)
            sd_ps = spsp.tile([128, B], dt.float32, tag="sdps")
            for q in range(2):
                for kh in range(4):
                    g = 4 * q + kh
                    nc.tensor.matmul(
                        sd_ps[:],
                        dmask_sb[:],
                        mt[g][:],
                        start=(kh == 0),
                        stop=(kh == 3),
                    )
                nc.scalar.mul(sneg[q][:], sd_ps[:], -0.25)

        c_acc = cpool.tile([128, IB], dt.float32, tag="cacc")
        c2 = cpool.tile([128, 160], dt.float32, tag="c2")
        # shifted band store: row i's exp window (cols [i+1, i+129)) lands at
        # flat [i*160 + i+1, i*160 + i+129); gaps stay zero so one strided
        # reduce over i yields the column-part sums
        e_all = cpool.tile([128, IB * 160], dt.bfloat16, tag="eall")
        nc.gpsimd.memset(e_all[:], 0.0)

        with tc.tile_pool(name="dps", bufs=2, space="PSUM") as dps, tc.tile_pool(
            name="eps", bufs=2, space="PSUM"
        ) as eps:
            WIN = 128  # circular half-window: row i covers j in (i, i+128]
            first_mm = True
            for i in range(IB):
                act_row = i % 6 == 5  # rows computed via ScalarE Abs (balance)
                lo, hi = i + 1, i + 1 + WIN
                d = dpool.tile([128, G * WIN], dt.bfloat16, tag="d")
                for g in range(G):
                    if act_row:
                        nc.scalar.activation(
                            d[:, g * WIN : (g + 1) * WIN],
                            mt[g][:, lo:hi],
                            Act.Abs,
                            bias=nmtf[g][:, i : i + 1],
                        )
                    else:
                        nc.vector.tensor_scalar(
                            d[:, g * WIN : (g + 1) * WIN],
                            mt[g][:, lo:hi],
                            mtf[g][:, i : i + 1],
                            0.0,
                            Alu.subtract,
                            Alu.max,
                        )
                diffs = dps.tile([128, WIN], dt.float32, tag="diffs")
                for q in range(2):
                    mms = []
                    if not act_row:
                        # seed the PSUM quadrant with -S_j/2
                        mms.append(
                            nc.tensor.matmul(
                                diffs[64 * q : 64 * (q + 1), :],
                                mask_sb[:],
                                sneg[q][:, lo:hi],
                                start=True,
                                stop=False,
                                skip_group_check=True,
                            )
                        )
                    for kh in range(4):
                        g = 4 * q + kh
                        mms.append(
                            nc.tensor.matmul(
                                diffs[64 * q : 64 * (q + 1), :],
                                mask_sb[:],
                                d[:, g * WIN : (g + 1) * WIN],
                                start=(act_row and kh == 0),
                                stop=(kh == 3),
                                skip_group_check=True,
                            )
                        )
                    for mm in mms:
                        # every pairwise matmul reuses the identical mask
                        # weights: skip the LDWEIGHTS reload after the first
                        if not first_mm:
                            mm.ins.ldweights = False
                        first_mm = False
                nc.scalar.activation(
                    e_all[:, i * 160 + lo : i * 160 + hi],
                    diffs[:],
                    Act.Exp,
                    scale=(-1.0 if act_row else -2.0),
                    bias=(0.0 if act_row else neg_s_bias[:, i : i + 1]),
                    accum_out=c_acc[:, i : i + 1],
                )

        # column-part: c2[o, c] = sum_i e_all[o, i*160 + c]
        e_view = e_all[:].rearrange("p (i c) -> p i c", i=IB)
        nc.vector.tensor_reduce(
            c2[:],
            e_view.rearrange("p i c -> p c i"),
            axis=mybir.AxisListType.X,
            op=Alu.add,
        )
        nc.sync.dma_start(cout, c_acc[:])
        nc.sync.dma_start(cout2, c2[:])

    nc.compile()
    return nc


def _prep_inputs(x, T):
    bf16 = ml_dtypes.bfloat16
    # permute W columns to (q, kh, o%64, k%2) so every per-group lhsT slice of
    # the GEMM is contiguous (walrus: weights AP must have one free dim)
    Wp = (
        np.asarray(T, np.float32)
        .reshape(F, 2, 64, 4, 2)
        .transpose(0, 1, 3, 2, 4)
        .reshape(F, O * K)
    )
    fp8 = ml_dtypes.float8_e4m3
    # SBUF-image interleave: row p holds [chunk0 | chunk1 | ...] where
    # chunk c covers input features [128c, 128c+128)
    W = np.ascontiguousarray(
        Wp.reshape(F // 128, 128, O * K).transpose(1, 0, 2).reshape(128, -1)
    ).astype(fp8)
    xTf = x.T.astype(np.float32)
    mask = np.zeros((128, 64), dtype=bf16)
    mask[np.arange(128), np.arange(128) // 2] = 1.0
    dmask = np.zeros((128, 128), dtype=bf16)
    p = np.arange(128)
    dmask[:, :] = (p[:, None] // 2 == p[None, :] // 2).astype(bf16)
    in_maps = []
    for b in range(NCORES):
        in_maps.append(
            {
                "w": W,
                "mask": mask,
                "dmask": dmask,
                "xt": np.ascontiguousarray(
                    np.roll(xTf, -IB * b, axis=1)
                    .reshape(F // 128, 128, B)
                    .transpose(1, 0, 2)
                    .reshape(128, -1)
                ).astype(fp8),
            }
        )
    return in_maps


def _assemble(x, results):
    # row-part from each core + banded column-part contributions from the
    # core itself and its four predecessors (window j in (i, i+128] wraps
    # across up to 5 row-blocks)
    c_full = np.zeros((B, O), np.float32)
    r = np.arange(IB)
    for b in range(NCORES):
        acc = results[b]["c"].astype(np.float32)  # (O, IB)
        rows = IB * b + r
        c_full[rows] = acc.T
        for t in range(5):
            c2 = results[(b - t) % NCORES]["c2"].astype(np.float32)  # (O, 160)
            c_full[rows] += c2[:, 32 * t + r].T
    return np.concatenate(
        [np.asarray(x, np.float32), c_full], axis=1
    )


def _get_nc():
    if "nc" not in _cache:
        _cache["nc"] = _build()
    return _cache["nc"]


def kernel(x, T):
    from concourse.bass_utils import run_bass_kernel_spmd

    x = np.asarray(x)
    T = np.asarray(T)
    nc = _get_nc()
    res = run_bass_kernel_spmd(nc, _prep_inputs(x, T), list(range(NCORES)))
    return _assemble(x, res.results)


def run_traced(x, T, **kwargs):
    """Like kernel() but returns (output, BassKernelResults) with tracing on."""
    from concourse.bass_utils import run_bass_kernel_spmd

    x = np.asarray(x)
    T = np.asarray(T)
    nc = _get_nc()
    res = run_bass_kernel_spmd(
        nc, _prep_inputs(x, T), list(range(NCORES)), trace=True, **kwargs
    )
    return _assemble(x, res.results), res
